# revision 40
# baseline (speedup 1.0000x reference)
"""CalderaLinear Trainium2 kernel (all-fp8 DMA + fp8 DoubleRow xr phase).

Computes out = x @ dequant(q).T + (x @ dequant(r).T) @ dequant(l).T + bias
with groupwise (group=128) dequantization, distributed over 8 NeuronCores
by sharding tokens (batch*seq) 8 ways and replicating the weights.

Numerics: the output scale is dominated by the low-rank path (|out| up to
~1.4e6), whose dominant component is rank-1-ish: c[n] = sum_k x[n,k]
amplified by r's positive mean. Quantizing x to fp8 puts ~3.6% noise on
c[n], so r is mean-centered on the host (r = r' + mu_j): the device
computes x8 @ r'8.T in fp8 DoubleRow (zero-mean r' kills the common-mode
amplification) and adds the rank-1 term mu_j * s[n] back via a K=1
matmul, with s[n] = sum_k x[n,k] computed exactly on the host. The xr
result and l stay bf16 for the closer GEMM (fp8 l puts a fixed per-output
error pattern on the large common component: measured 1.9e-2 absmax).
Simulated pipeline error: 6.0e-3 absmax (gate 2e-2).

Host does layout only: dequant-multiply + mean-center + transposes +
fp8/bf16 casts + per-token sums + token sharding.

Device per core (1024 tokens):
  x arrives directly as fp8 [P, G, T] (g-major), fetched in g-pair slices
  so the first output-block pair's matmuls start after ~2 DMA slices
  instead of the full tensor. The startup phase runs gp-major across
  token tiles 0-2 (6 PSUM banks) so each 512KB of DMA unlocks 6 matmuls.
  A short burst of N=64 warmup matmuls on a zeroed tile covers the
  initial DMA window and lifts the PE HAM clock-gate (1.2->2.4 GHz)
  before real work lands. Engines execute in emission order, so emission
  order here IS the schedule.
"""

import os
import sys

import numpy as np
import ml_dtypes

for _p in ("/opt/trn_rl_repo",):
    if _p not in sys.path and os.path.isdir(_p):
        sys.path.insert(0, _p)

import concourse.bass as bass
import concourse.mybir as mybir
import concourse.tile as tile
from concourse import bacc
from concourse.bass_utils import run_bass_kernel_spmd

BF16 = mybir.dt.bfloat16
F32 = mybir.dt.float32
FP8 = mybir.dt.float8e4
NP_FP8 = ml_dtypes.float8_e4m3
NP_BF16 = ml_dtypes.bfloat16

P = 128  # partitions / dequant group size
N_CORES = 8

# Full problem shape (hardcoded per contest contract).
B, S, D_IN, D_OUT, RANK = 4, 2048, 4096, 4096, 256
N_TOK = B * S          # 8192
T = N_TOK // N_CORES   # 1024 tokens per core
G = D_IN // P          # 32 k-chunks
GP = G // 2            # 16 k-pair-chunks (DoubleRow)
OBW = 512              # output block width
NOB = D_OUT // OBW     # 8 output blocks
RC = RANK // P         # 2 rank chunks
NT = T // P            # 8 token tiles
N_WARMUP = 64          # HAM warmup matmuls (N=64 each) before first data
# Warmup bursts woven between early gp-steps: insurance against DMA
# hiccups re-throttling the HAM clock-gate; taper to zero once DMA leads.
WARM_WEAVE = [8, 6, 4, 2, 0, 0, 0, 0, 0, 0, 0, 0, 0, 0, 0, 0]


def caldera_kernel(tc, out, x8_d, q8_d, rp8_d, lT_d, smu_d):
    """One core. DRAM tensors:
    x8_d    [P, G, T]            fp8   x8[p,g,t] = x[t, g*128+p]
    q8_d    [NOB, 128, GP*2*OBW] fp8   q8[ob,p,(gp,i,o)] =
                                         qdeq[(2gp+i)*128+p, ob*512+o]
    rp8_d   [P, G, RANK]         fp8   rp8[p,g,r] = (rdeq-mu)[r, g*128+p]
    lT_d    [128, RC, D_OUT]     bf16  lT[p,c,o] = ldeq[o, c*128+p]
    smu_d   [1, T+RANK]          bf16  [s[0:T] | mu[0:RANK]]
    out     [T, D_OUT]           bf16  (bias added on host)
    """
    nc = tc.nc
    DR = mybir.MatmulPerfMode.DoubleRow

    with tc.tile_pool(name="const", bufs=1) as constp, \
         tc.tile_pool(name="qsE", bufs=3) as qsE, \
         tc.tile_pool(name="outp", bufs=6) as outp, \
         tc.tile_pool(name="otwp", bufs=1) as otwp, \
         tc.tile_pool(name="ps", bufs=8, space="PSUM") as psp:

        # ---- resident tensors ----
        x8 = constp.tile([P, G, T], FP8)
        rp8 = constp.tile([P, G, RANK], FP8)
        lT = constp.tile([P, RC, D_OUT], BF16)
        xrT = constp.tile([P, RC, T], BF16)     # xr.T chunks (bf16)
        smu = constp.tile([P, T + RANK], BF16)  # row 0 only: [s | mu]
        warm = constp.tile([P, P], FP8)         # HAM warmup garbage

        qtiles = {}

        def fetch_q(ob, pool, frac=None):
            if ob not in qtiles:
                qtiles[ob] = pool.tile(
                    [P, GP, 2, OBW], FP8, tag="q8b", name=f"q8b{ob}"
                )
            qt = qtiles[ob]
            flat = qt[:].rearrange("p a b c -> p (a b c)")
            if frac is None:
                nc.sync.dma_start(out=flat, in_=q8_d[ob])
            else:
                i, n = frac
                h = GP * 2 * OBW // n
                nc.sync.dma_start(
                    out=flat[:, i * h:(i + 1) * h],
                    in_=q8_d[ob][:, i * h:(i + 1) * h],
                )

        def fetch_x(glo, ghi):
            nc.sync.dma_start(
                out=x8[:, glo:ghi, :], in_=x8_d[:, glo:ghi, :],
            )

        def open_gp(ps_pair, t, qA, qB, gp):
            """One DoubleRow k-pair step of the main GEMM for tile t."""
            psA, psB = ps_pair
            lhs = x8[:, 2 * gp:2 * gp + 2, t * P:(t + 1) * P]
            nc.tensor.matmul(
                psA[:], lhsT=lhs, rhs=qA[:, gp],
                start=(gp == 0), stop=False, perf_mode=DR,
            )
            nc.tensor.matmul(
                psB[:], lhsT=lhs, rhs=qB[:, gp],
                start=(gp == 0), stop=False, perf_mode=DR,
            )

        def main_group_open(ps_pair, t, qA, qB):
            for gp in range(GP):
                open_gp(ps_pair, t, qA, qB, gp)

        drain_tog = [0]

        def close_fused(ps_pair, t, obA, obB):
            """Final-group close: both obs drain (scalar ∥ vector) into
            one wide tile, stored with a single DMA issue."""
            psA, psB = ps_pair
            for ps in (psA, psB):
                ob = obA if ps is psA else obB
                for rb in range(RC):
                    nc.tensor.matmul(
                        ps[:], lhsT=xrT[:, rb, t * P:(t + 1) * P],
                        rhs=lT[:, rb, ob * OBW:(ob + 1) * OBW],
                        start=False, stop=(rb == RC - 1),
                    )
            otw = otwp.tile([P, 2 * OBW], BF16, tag="otw")
            nc.scalar.copy(otw[:, 0:OBW], psA[:])
            nc.vector.tensor_copy(out=otw[:, OBW:2 * OBW], in_=psB[:])
            nc.sync.dma_start(
                out=out[t * P:(t + 1) * P, obA * OBW:(obB + 1) * OBW],
                in_=otw[:],
            )

        def main_group_close(ps_pair, t, obA, obB):
            """Low-rank closers (bf16) + psum drain + store.

            Bias is added on the host, so the drain is a plain
            fp32->bf16 copy; alternating it between the Scalar and
            Vector engines halves the serialized drain chain."""
            psA, psB = ps_pair
            for ps, ob in ((psA, obA), (psB, obB)):
                for rb in range(RC):
                    nc.tensor.matmul(
                        ps[:], lhsT=xrT[:, rb, t * P:(t + 1) * P],
                        rhs=lT[:, rb, ob * OBW:(ob + 1) * OBW],
                        start=False, stop=(rb == RC - 1),
                    )
                ot = outp.tile([P, OBW], BF16, tag="ot")
                if drain_tog[0] % 2 == 0:
                    nc.scalar.copy(ot[:], ps[:])
                else:
                    nc.vector.tensor_copy(out=ot[:], in_=ps[:])
                drain_tog[0] += 1
                nc.sync.dma_start(
                    out=out[t * P:(t + 1) * P, ob * OBW:(ob + 1) * OBW],
                    in_=ot[:],
                )

        def new_pair(name):
            psA = psp.tile([P, OBW], F32, tag="ps", name=f"psA{name}")
            psB = psp.tile([P, OBW], F32, tag="ps", name=f"psB{name}")
            return psA, psB

        def xr_alloc(th):
            return [
                psp.tile([P, OBW], F32, tag="ps", name=f"xrps{th}_{rb}")
                for rb in range(RC)
            ]

        def xr_open(pss, th, gplo, gphi):
            """xr accumulation (fp8 DoubleRow) for k-pairs [gplo, gphi)."""
            tok = slice(th * OBW, (th + 1) * OBW)
            for gp in range(gplo, gphi):
                for rb in range(RC):
                    nc.tensor.matmul(
                        pss[rb][:],
                        lhsT=rp8[:, 2 * gp:2 * gp + 2, rb * P:(rb + 1) * P],
                        rhs=x8[:, 2 * gp:2 * gp + 2, tok],
                        start=(gp == 0), stop=False, perf_mode=DR,
                    )

        def xr_close(pss, th):
            """Rank-1 mu[j]*s[n] term (K=1 bf16 matmul) + psum->bf16."""
            tok = slice(th * OBW, (th + 1) * OBW)
            for rb in range(RC):
                nc.tensor.matmul(
                    pss[rb][:],
                    lhsT=smu[0:1, T + rb * P:T + (rb + 1) * P],
                    rhs=smu[0:1, tok],
                    start=False, stop=True,
                )
            for rb in range(RC):
                nc.scalar.copy(xrT[:, rb, tok], pss[rb][:])

        # ================= DMA emission order = fetch priority ============
        # Each dma_start costs ~650ns of serial issue time on the Sync
        # engine, so effective bandwidth is proportional to DMA size for
        # small transfers. Use fine slices only for the first two
        # gp-steps (fast first matmul), then 512KB-1MB chunks in strict
        # consumption order. Bulk tensors (rp8, smu, lT, biasr, q2) slot
        # in after the gp-steps they'd otherwise delay.
        def fetch_rp8(qlo, qhi):
            nc.sync.dma_start(
                out=rp8[:, qlo:qhi, :], in_=rp8_d[:, qlo:qhi, :],
            )

        fetch_x(0, 2)                     # gp 0
        fetch_q(0, qsE, frac=(0, 16))
        fetch_q(1, qsE, frac=(0, 16))
        fetch_x(2, 4)                     # gp 1
        fetch_q(0, qsE, frac=(1, 16))
        fetch_q(1, qsE, frac=(1, 16))
        fetch_rp8(0, 8)
        fetch_x(4, 8)                     # gp 2-3
        fetch_q(0, qsE, frac=(1, 8))
        fetch_q(1, qsE, frac=(1, 8))
        fetch_rp8(8, 16)
        fetch_x(8, 16)                    # gp 4-7
        fetch_q(0, qsE, frac=(1, 4))
        fetch_q(1, qsE, frac=(1, 4))
        fetch_rp8(16, 24)
        fetch_x(16, 24)                   # gp 8-11
        fetch_q(0, qsE, frac=(2, 4))
        fetch_q(1, qsE, frac=(2, 4))
        fetch_rp8(24, 32)
        fetch_x(24, 32)                   # gp 12-15
        fetch_q(0, qsE, frac=(3, 4))
        fetch_q(1, qsE, frac=(3, 4))
        nc.sync.dma_start(out=smu[0:1, :], in_=smu_d[:])
        nc.sync.dma_start(out=lT[:, :, 0:2 * OBW], in_=lT_d[:, :, 0:2 * OBW])
        nc.sync.dma_start(
            out=lT[:, :, 2 * OBW:D_OUT], in_=lT_d[:, :, 2 * OBW:D_OUT]
        )
        fetch_q(2, qsE)

        # ================= PE emission order = schedule ===================
        # HAM warmup: keep the PE busy through the first DMA window.
        # Vector engine is ready ~2us before GpSimd, so memset there.
        nc.vector.memset(warm[:], 0.0)
        wps = psp.tile([P, OBW], F32, tag="ps", name="warmps")

        def warmup(n):
            for _ in range(n):
                nc.tensor.matmul(
                    wps[:, 0:64], lhsT=warm[:, 0:P], rhs=warm[:, 0:64],
                    start=True, stop=True,
                )

        warmup(N_WARMUP)
        qA, qB = qtiles[0], qtiles[1]
        # gp-major across tiles 0-1 (4 banks) with the xr-half-0
        # accumulation woven in two gp-steps behind (it reuses x slices
        # already fetched for the opens, so it is stall-free PE filler
        # while DMA paces the startup). wps(1)+pairs(4)+xr(2) <= 8 banks.
        pairs = {t: new_pair(f"0_{t}") for t in (0, 1)}
        xr0 = xr_alloc(0)
        for gp in range(GP):
            for t in (0, 1):
                open_gp(pairs[t], t, qA, qB, gp)
            warmup(WARM_WEAVE[gp])
            if gp >= 2:
                xr_open(xr0, 0, gp - 2, gp - 1)
        xr_open(xr0, 0, GP - 2, GP)
        xr_close(xr0, 0)
        for t in (0, 1):
            main_group_close(pairs[t], t, 0, 1)
        # Batch opens before closes: each DoubleRow<->bf16 PE mode switch
        # costs ~200ns on the first matmul after it, so group the bf16
        # closers of several tiles together (PSUM budget permitting).
        for t in (2, 3, 4):
            pairs[t] = new_pair(f"0_{t}")
            main_group_open(pairs[t], t, qA, qB)
        xr1 = xr_alloc(1)               # 2 banks (6 pair + 2 xr = 8)
        xr_open(xr1, 1, 0, GP)
        xr_close(xr1, 1)
        for t in (2, 3, 4):
            main_group_close(pairs[t], t, 0, 1)
        for t in (5, 6, 7):
            pairs[t] = new_pair(f"0_{t}")
            main_group_open(pairs[t], t, qA, qB)
        for t in (5, 6, 7):
            main_group_close(pairs[t], t, 0, 1)

        # ================= phase 2: remaining block pairs =================
        with tc.tile_pool(name="qsL", bufs=5) as qsL:
            for ob in range(3, NOB):
                fetch_q(ob, qsL)
            for obp in range(1, NOB // 2):
                obA, obB = 2 * obp, 2 * obp + 1
                qA, qB = qtiles[obA], qtiles[obB]
                # Last block pair tapers its batches so the final group's
                # 2 store issues (~650ns each, serialized on Sync) are
                # all that lands after the last matmul.
                if obp == NOB // 2 - 1:
                    batches = [(0, 1, 2, 3), (4, 5, 6), (7,)]
                else:
                    batches = [(0, 1, 2, 3), (4, 5, 6, 7)]
                for tb in batches:
                    pps = [new_pair(f"{obp}_{t}") for t in tb]
                    for i, t in enumerate(tb):
                        main_group_open(pps[i], t, qA, qB)
                    for i, t in enumerate(tb):
                        if tb == (7,):
                            close_fused(pps[i], t, obA, obB)
                        else:
                            main_group_close(pps[i], t, obA, obB)


def build_nc():
    nc = bacc.Bacc("TRN2", target_bir_lowering=False, debug=False)
    x8_d = nc.dram_tensor("x8", [P, G, T], FP8, kind="ExternalInput").ap()
    q8_d = nc.dram_tensor(
        "q8", [NOB, P, GP * 2 * OBW], FP8, kind="ExternalInput"
    ).ap()
    rp8_d = nc.dram_tensor(
        "rp8", [P, G, RANK], FP8, kind="ExternalInput"
    ).ap()
    lT_d = nc.dram_tensor("lT", [P, RC, D_OUT], BF16, kind="ExternalInput").ap()
    smu_d = nc.dram_tensor(
        "smu", [1, T + RANK], BF16, kind="ExternalInput"
    ).ap()
    out = nc.dram_tensor("out", [T, D_OUT], BF16, kind="ExternalOutput").ap()
    with tile.TileContext(nc) as tc:
        caldera_kernel(tc, out, x8_d, q8_d, rp8_d, lT_d, smu_d)
    nc.compile()
    return nc


def _dequant(vals, scales):
    rows, cols = vals.shape
    g = cols // P
    v = vals.astype(np.float32).reshape(rows, g, P) * scales[:, :, None]
    return v.reshape(rows, cols)


def make_in_maps(x, q_values, q_scales, l_values, l_scales, r_values, r_scales,
                 bias):
    # q: dequant -> [k, o] transpose -> fp8, packed per 512-col block:
    # q8[ob, p, (gp, i, o)] = qdeq[(2gp+i)*128+p, ob*512+o]
    qdeq = _dequant(np.asarray(q_values), np.asarray(q_scales))  # [o, k]
    qT = np.ascontiguousarray(qdeq.T).astype(NP_FP8)             # [k, o]
    q8 = qT.reshape(GP, 2, P, NOB, OBW).transpose(3, 2, 0, 1, 4)
    q8 = np.ascontiguousarray(q8).reshape(NOB, P, GP * 2 * OBW)

    rdeq = _dequant(np.asarray(r_values), np.asarray(r_scales))  # [r, k]
    mu = rdeq.mean(axis=1).astype(np.float32)                    # [r]
    rp = rdeq - mu[:, None]                                      # zero-mean
    rp8 = np.ascontiguousarray(
        rp.T.reshape(G, P, RANK).transpose(1, 0, 2)
    ).astype(NP_FP8)                                             # [p, g, r]

    ldeq = _dequant(np.asarray(l_values), np.asarray(l_scales))  # [o, r]
    lT = np.ascontiguousarray(
        ldeq.T.reshape(RC, P, D_OUT).transpose(1, 0, 2)
    ).astype(NP_BF16)                                            # [p, c, o]

    xf = np.asarray(x, dtype=np.float32).reshape(N_TOK, D_IN)
    s_all = xf.sum(axis=1)                                       # [n_tok]
    in_maps = []
    for i in range(N_CORES):
        xs = xf[i * T:(i + 1) * T]                               # [t, k]
        x8 = np.ascontiguousarray(
            xs.reshape(T, G, P).transpose(2, 1, 0)
        ).astype(NP_FP8)                                         # [p, g, t]
        smu = np.concatenate([s_all[i * T:(i + 1) * T], mu]).astype(
            NP_BF16
        ).reshape(1, T + RANK)
        in_maps.append({
            "x8": x8, "q8": q8, "rp8": rp8, "lT": lT, "smu": smu,
        })
    return in_maps


_NC_CACHE = {}


def _get_nc():
    if "nc" not in _NC_CACHE:
        _NC_CACHE["nc"] = build_nc()
    return _NC_CACHE["nc"]


def run(inputs, trace=False, tmpdir=None):
    nc = _get_nc()
    in_maps = make_in_maps(**inputs)
    res = run_bass_kernel_spmd(
        nc, in_maps, list(range(N_CORES)), trace=trace, tmpdir=tmpdir
    )
    shards = [
        np.asarray(res.results[i]["out"]).astype(np.float32)
        for i in range(N_CORES)
    ]
    full = np.concatenate(shards, axis=0)
    full += np.asarray(inputs["bias"], dtype=np.float32)
    return full.reshape(B, S, D_OUT), res


def kernel(**inputs) -> np.ndarray:
    out, _ = run(inputs, trace=False)
    return out


# revision 43
# speedup vs baseline: 1.0067x; 1.0067x over previous
"""CalderaLinear Trainium2 kernel (all-fp8 DMA + fp8 DoubleRow xr phase).

Computes out = x @ dequant(q).T + (x @ dequant(r).T) @ dequant(l).T + bias
with groupwise (group=128) dequantization, distributed over 8 NeuronCores
by sharding tokens (batch*seq) 8 ways and replicating the weights.

Numerics: the output scale is dominated by the low-rank path (|out| up to
~1.4e6), whose dominant component is rank-1-ish: c[n] = sum_k x[n,k]
amplified by r's positive mean. Quantizing x to fp8 puts ~3.6% noise on
c[n], so r is mean-centered on the host (r = r' + mu_j): the device
computes x8 @ r'8.T in fp8 DoubleRow (zero-mean r' kills the common-mode
amplification) and adds the rank-1 term mu_j * s[n] back via a K=1
matmul, with s[n] = sum_k x[n,k] computed exactly on the host. The xr
result and l stay bf16 for the closer GEMM (fp8 l puts a fixed per-output
error pattern on the large common component: measured 1.9e-2 absmax).
Simulated pipeline error: 6.0e-3 absmax (gate 2e-2).

Host does layout only: dequant-multiply + mean-center + transposes +
fp8/bf16 casts + per-token sums + token sharding.

Device per core (1024 tokens):
  x arrives directly as fp8 [P, G, T] (g-major), fetched in g-pair slices
  so the first output-block pair's matmuls start after ~2 DMA slices
  instead of the full tensor. The startup phase runs gp-major across
  token tiles 0-2 (6 PSUM banks) so each 512KB of DMA unlocks 6 matmuls.
  A short burst of N=64 warmup matmuls on a zeroed tile covers the
  initial DMA window and lifts the PE HAM clock-gate (1.2->2.4 GHz)
  before real work lands. Engines execute in emission order, so emission
  order here IS the schedule.
"""

import os
import sys

import numpy as np
import ml_dtypes

for _p in ("/opt/trn_rl_repo",):
    if _p not in sys.path and os.path.isdir(_p):
        sys.path.insert(0, _p)

import concourse.bass as bass
import concourse.mybir as mybir
import concourse.tile as tile
from concourse import bacc
from concourse.bass_utils import run_bass_kernel_spmd

BF16 = mybir.dt.bfloat16
F32 = mybir.dt.float32
FP8 = mybir.dt.float8e4
NP_FP8 = ml_dtypes.float8_e4m3
NP_BF16 = ml_dtypes.bfloat16

P = 128  # partitions / dequant group size
N_CORES = 8

# Full problem shape (hardcoded per contest contract).
B, S, D_IN, D_OUT, RANK = 4, 2048, 4096, 4096, 256
N_TOK = B * S          # 8192
T = N_TOK // N_CORES   # 1024 tokens per core
G = D_IN // P          # 32 k-chunks
GP = G // 2            # 16 k-pair-chunks (DoubleRow)
OBW = 512              # output block width
NOB = D_OUT // OBW     # 8 output blocks
RC = RANK // P         # 2 rank chunks
NT = T // P            # 8 token tiles
N_WARMUP = 64          # HAM warmup matmuls (N=64 each) before first data
# Warmup bursts woven between early gp-steps: insurance against DMA
# hiccups re-throttling the HAM clock-gate; taper to zero once DMA leads.
# Only gp 0-1 may weave warmups: they share xr0's PSUM bank, whose real
# accumulation (start=True) begins at gp==2.
WARM_WEAVE = [8, 6] + [0] * 14


def caldera_kernel(tc, out, x8_d, q8_d, rp8_d, lT_d, smu_d):
    """One core. DRAM tensors:
    x8_d    [P, G, T]            fp8   x8[p,g,t] = x[t, g*128+p]
    q8_d    [NOB, 128, GP*2*OBW] fp8   q8[ob,p,(gp,i,o)] =
                                         qdeq[(2gp+i)*128+p, ob*512+o]
    rp8_d   [P, G, RANK]         fp8   rp8[p,g,r] = (rdeq-mu)[r, g*128+p]
    lT_d    [128, RC, D_OUT]     bf16  lT[p,c,o] = ldeq[o, c*128+p]
    smu_d   [1, T+RANK]          bf16  [s[0:T] | mu[0:RANK]]
    out     [T, D_OUT]           bf16  (bias added on host)
    """
    nc = tc.nc
    DR = mybir.MatmulPerfMode.DoubleRow

    with tc.tile_pool(name="const", bufs=1) as constp, \
         tc.tile_pool(name="qsE", bufs=3) as qsE, \
         tc.tile_pool(name="outp", bufs=6) as outp, \
         tc.tile_pool(name="otwp", bufs=1) as otwp, \
         tc.tile_pool(name="ps", bufs=8, space="PSUM") as psp:

        # ---- resident tensors ----
        x8 = constp.tile([P, G, T], FP8)
        rp8 = constp.tile([P, G, RANK], FP8)
        lT = constp.tile([P, RC, D_OUT], BF16)
        xrT = constp.tile([P, RC, T], BF16)     # xr.T chunks (bf16)
        smu = constp.tile([P, T + RANK], BF16)  # row 0 only: [s | mu]
        warm = constp.tile([P, P], FP8)         # HAM warmup garbage

        qtiles = {}

        def fetch_q(ob, pool, frac=None):
            if ob not in qtiles:
                qtiles[ob] = pool.tile(
                    [P, GP, 2, OBW], FP8, tag="q8b", name=f"q8b{ob}"
                )
            qt = qtiles[ob]
            flat = qt[:].rearrange("p a b c -> p (a b c)")
            if frac is None:
                nc.sync.dma_start(out=flat, in_=q8_d[ob])
            else:
                i, n = frac
                h = GP * 2 * OBW // n
                nc.sync.dma_start(
                    out=flat[:, i * h:(i + 1) * h],
                    in_=q8_d[ob][:, i * h:(i + 1) * h],
                )

        def fetch_x(glo, ghi):
            nc.sync.dma_start(
                out=x8[:, glo:ghi, :], in_=x8_d[:, glo:ghi, :],
            )

        def open_gp(ps_pair, t, qA, qB, gp):
            """One DoubleRow k-pair step of the main GEMM for tile t."""
            psA, psB = ps_pair
            lhs = x8[:, 2 * gp:2 * gp + 2, t * P:(t + 1) * P]
            nc.tensor.matmul(
                psA[:], lhsT=lhs, rhs=qA[:, gp],
                start=(gp == 0), stop=False, perf_mode=DR,
            )
            nc.tensor.matmul(
                psB[:], lhsT=lhs, rhs=qB[:, gp],
                start=(gp == 0), stop=False, perf_mode=DR,
            )

        def main_group_open(ps_pair, t, qA, qB):
            for gp in range(GP):
                open_gp(ps_pair, t, qA, qB, gp)

        drain_tog = [0]

        def close_fused(ps_pair, t, obA, obB):
            """Final-group close: both obs drain (scalar ∥ vector) into
            one wide tile, stored with a single DMA issue."""
            psA, psB = ps_pair
            for ps in (psA, psB):
                ob = obA if ps is psA else obB
                for rb in range(RC):
                    nc.tensor.matmul(
                        ps[:], lhsT=xrT[:, rb, t * P:(t + 1) * P],
                        rhs=lT[:, rb, ob * OBW:(ob + 1) * OBW],
                        start=False, stop=(rb == RC - 1),
                    )
            otw = otwp.tile([P, 2 * OBW], BF16, tag="otw")
            nc.scalar.copy(otw[:, 0:OBW], psA[:])
            nc.vector.tensor_copy(out=otw[:, OBW:2 * OBW], in_=psB[:])
            nc.sync.dma_start(
                out=out[t * P:(t + 1) * P, obA * OBW:(obB + 1) * OBW],
                in_=otw[:],
            )

        def main_group_close(ps_pair, t, obA, obB):
            """Low-rank closers (bf16) + psum drain + store.

            Bias is added on the host, so the drain is a plain
            fp32->bf16 copy; alternating it between the Scalar and
            Vector engines halves the serialized drain chain."""
            psA, psB = ps_pair
            for ps, ob in ((psA, obA), (psB, obB)):
                for rb in range(RC):
                    nc.tensor.matmul(
                        ps[:], lhsT=xrT[:, rb, t * P:(t + 1) * P],
                        rhs=lT[:, rb, ob * OBW:(ob + 1) * OBW],
                        start=False, stop=(rb == RC - 1),
                    )
                ot = outp.tile([P, OBW], BF16, tag="ot")
                if drain_tog[0] % 2 == 0:
                    nc.scalar.copy(ot[:], ps[:])
                else:
                    nc.vector.tensor_copy(out=ot[:], in_=ps[:])
                drain_tog[0] += 1
                nc.sync.dma_start(
                    out=out[t * P:(t + 1) * P, ob * OBW:(ob + 1) * OBW],
                    in_=ot[:],
                )

        def new_pair(name):
            psA = psp.tile([P, OBW], F32, tag="ps", name=f"psA{name}")
            psB = psp.tile([P, OBW], F32, tag="ps", name=f"psB{name}")
            return psA, psB

        def xr_alloc(th):
            return [
                psp.tile([P, OBW], F32, tag="ps", name=f"xrps{th}_{rb}")
                for rb in range(RC)
            ]

        def xr_open(pss, th, gplo, gphi):
            """xr accumulation (fp8 DoubleRow) for k-pairs [gplo, gphi)."""
            tok = slice(th * OBW, (th + 1) * OBW)
            for gp in range(gplo, gphi):
                for rb in range(RC):
                    nc.tensor.matmul(
                        pss[rb][:],
                        lhsT=rp8[:, 2 * gp:2 * gp + 2, rb * P:(rb + 1) * P],
                        rhs=x8[:, 2 * gp:2 * gp + 2, tok],
                        start=(gp == 0), stop=False, perf_mode=DR,
                    )

        def xr_close(pss, th):
            """Rank-1 mu[j]*s[n] term (K=1 bf16 matmul) + psum->bf16."""
            tok = slice(th * OBW, (th + 1) * OBW)
            for rb in range(RC):
                nc.tensor.matmul(
                    pss[rb][:],
                    lhsT=smu[0:1, T + rb * P:T + (rb + 1) * P],
                    rhs=smu[0:1, tok],
                    start=False, stop=True,
                )
            for rb in range(RC):
                nc.scalar.copy(xrT[:, rb, tok], pss[rb][:])

        # ================= DMA emission order = fetch priority ============
        # Each dma_start costs ~650ns of serial issue time on the Sync
        # engine, so effective bandwidth is proportional to DMA size for
        # small transfers. Use fine slices only for the first two
        # gp-steps (fast first matmul), then 512KB-1MB chunks in strict
        # consumption order. Bulk tensors (rp8, smu, lT, biasr, q2) slot
        # in after the gp-steps they'd otherwise delay.
        def fetch_rp8(qlo, qhi):
            nc.sync.dma_start(
                out=rp8[:, qlo:qhi, :], in_=rp8_d[:, qlo:qhi, :],
            )

        fetch_x(0, 2)                     # gp 0
        fetch_q(0, qsE, frac=(0, 16))
        fetch_q(1, qsE, frac=(0, 16))
        fetch_x(2, 4)                     # gp 1
        fetch_q(0, qsE, frac=(1, 16))
        fetch_q(1, qsE, frac=(1, 16))
        fetch_rp8(0, 8)
        fetch_x(4, 8)                     # gp 2-3
        fetch_q(0, qsE, frac=(1, 8))
        fetch_q(1, qsE, frac=(1, 8))
        fetch_rp8(8, 16)
        fetch_x(8, 16)                    # gp 4-7
        fetch_q(0, qsE, frac=(1, 4))
        fetch_q(1, qsE, frac=(1, 4))
        fetch_rp8(16, 24)
        fetch_x(16, 24)                   # gp 8-11
        fetch_q(0, qsE, frac=(2, 4))
        fetch_q(1, qsE, frac=(2, 4))
        fetch_rp8(24, 32)
        fetch_x(24, 32)                   # gp 12-15
        fetch_q(0, qsE, frac=(3, 4))
        fetch_q(1, qsE, frac=(3, 4))
        nc.sync.dma_start(out=smu[0:1, :], in_=smu_d[:])
        nc.sync.dma_start(out=lT[:, :, 0:2 * OBW], in_=lT_d[:, :, 0:2 * OBW])
        nc.sync.dma_start(
            out=lT[:, :, 2 * OBW:D_OUT], in_=lT_d[:, :, 2 * OBW:D_OUT]
        )
        fetch_q(2, qsE)

        # ================= PE emission order = schedule ===================
        # HAM warmup: keep the PE busy through the first DMA window.
        # Vector engine is ready ~2us before GpSimd, so memset there.
        # Warmup matmuls borrow xr0's PSUM bank (set below) — its real
        # accumulation later opens with start=True, clearing them.
        nc.vector.memset(warm[:], 0.0)
        wps = None

        def warmup(n):
            for _ in range(n):
                nc.tensor.matmul(
                    wps[:, 0:64], lhsT=warm[:, 0:P], rhs=warm[:, 0:64],
                    start=True, stop=True,
                )

        qA, qB = qtiles[0], qtiles[1]
        # gp-major across tiles 0-2 (6 banks) with the xr-half-0
        # accumulation woven two gp-steps behind (it reuses x slices
        # already fetched for the opens, so it is stall-free PE filler
        # while DMA paces the startup). 6 pairs + 2 xr = 8 banks; the
        # warmups borrow xr0's bank before its gp==2 start=True clear.
        pairs = {t: new_pair(f"0_{t}") for t in (0, 1, 2)}
        xr0 = xr_alloc(0)
        wps = xr0[0]
        warmup(N_WARMUP)
        for gp in range(GP):
            for t in (0, 1, 2):
                open_gp(pairs[t], t, qA, qB, gp)
            warmup(WARM_WEAVE[gp])
            if gp >= 2:
                xr_open(xr0, 0, gp - 2, gp - 1)
        xr_open(xr0, 0, GP - 2, GP)
        xr_close(xr0, 0)
        for t in (0, 1, 2):
            main_group_close(pairs[t], t, 0, 1)
        # Batch opens before closes: each DoubleRow<->bf16 PE mode switch
        # costs ~200ns on the first matmul after it, so group the bf16
        # closers of several tiles together (PSUM budget permitting).
        for t in (3, 4):
            pairs[t] = new_pair(f"0_{t}")
            main_group_open(pairs[t], t, qA, qB)
        xr1 = xr_alloc(1)               # 2 banks (4 pair + 2 xr <= 8)
        xr_open(xr1, 1, 0, GP)
        xr_close(xr1, 1)
        for t in (3, 4):
            main_group_close(pairs[t], t, 0, 1)
        for t in (5, 6, 7):
            pairs[t] = new_pair(f"0_{t}")
            main_group_open(pairs[t], t, qA, qB)
        for t in (5, 6, 7):
            main_group_close(pairs[t], t, 0, 1)

        # ================= phase 2: remaining block pairs =================
        with tc.tile_pool(name="qsL", bufs=5) as qsL:
            for ob in range(3, NOB):
                fetch_q(ob, qsL)
            for obp in range(1, NOB // 2):
                obA, obB = 2 * obp, 2 * obp + 1
                qA, qB = qtiles[obA], qtiles[obB]
                # Last block pair tapers its batches so the final group's
                # 2 store issues (~650ns each, serialized on Sync) are
                # all that lands after the last matmul.
                if obp == NOB // 2 - 1:
                    batches = [(0, 1, 2, 3), (4, 5, 6), (7,)]
                else:
                    batches = [(0, 1, 2, 3), (4, 5, 6, 7)]
                for tb in batches:
                    pps = [new_pair(f"{obp}_{t}") for t in tb]
                    for i, t in enumerate(tb):
                        main_group_open(pps[i], t, qA, qB)
                    for i, t in enumerate(tb):
                        if tb == (7,):
                            close_fused(pps[i], t, obA, obB)
                        else:
                            main_group_close(pps[i], t, obA, obB)


def build_nc():
    nc = bacc.Bacc("TRN2", target_bir_lowering=False, debug=False)
    x8_d = nc.dram_tensor("x8", [P, G, T], FP8, kind="ExternalInput").ap()
    q8_d = nc.dram_tensor(
        "q8", [NOB, P, GP * 2 * OBW], FP8, kind="ExternalInput"
    ).ap()
    rp8_d = nc.dram_tensor(
        "rp8", [P, G, RANK], FP8, kind="ExternalInput"
    ).ap()
    lT_d = nc.dram_tensor("lT", [P, RC, D_OUT], BF16, kind="ExternalInput").ap()
    smu_d = nc.dram_tensor(
        "smu", [1, T + RANK], BF16, kind="ExternalInput"
    ).ap()
    out = nc.dram_tensor("out", [T, D_OUT], BF16, kind="ExternalOutput").ap()
    with tile.TileContext(nc) as tc:
        caldera_kernel(tc, out, x8_d, q8_d, rp8_d, lT_d, smu_d)
    nc.compile()
    return nc


def _dequant(vals, scales):
    rows, cols = vals.shape
    g = cols // P
    v = vals.astype(np.float32).reshape(rows, g, P) * scales[:, :, None]
    return v.reshape(rows, cols)


def make_in_maps(x, q_values, q_scales, l_values, l_scales, r_values, r_scales,
                 bias):
    # q: dequant -> [k, o] transpose -> fp8, packed per 512-col block:
    # q8[ob, p, (gp, i, o)] = qdeq[(2gp+i)*128+p, ob*512+o]
    qdeq = _dequant(np.asarray(q_values), np.asarray(q_scales))  # [o, k]
    qT = np.ascontiguousarray(qdeq.T).astype(NP_FP8)             # [k, o]
    q8 = qT.reshape(GP, 2, P, NOB, OBW).transpose(3, 2, 0, 1, 4)
    q8 = np.ascontiguousarray(q8).reshape(NOB, P, GP * 2 * OBW)

    rdeq = _dequant(np.asarray(r_values), np.asarray(r_scales))  # [r, k]
    mu = rdeq.mean(axis=1).astype(np.float32)                    # [r]
    rp = rdeq - mu[:, None]                                      # zero-mean
    rp8 = np.ascontiguousarray(
        rp.T.reshape(G, P, RANK).transpose(1, 0, 2)
    ).astype(NP_FP8)                                             # [p, g, r]

    ldeq = _dequant(np.asarray(l_values), np.asarray(l_scales))  # [o, r]
    lT = np.ascontiguousarray(
        ldeq.T.reshape(RC, P, D_OUT).transpose(1, 0, 2)
    ).astype(NP_BF16)                                            # [p, c, o]

    xf = np.asarray(x, dtype=np.float32).reshape(N_TOK, D_IN)
    s_all = xf.sum(axis=1)                                       # [n_tok]
    in_maps = []
    for i in range(N_CORES):
        xs = xf[i * T:(i + 1) * T]                               # [t, k]
        x8 = np.ascontiguousarray(
            xs.reshape(T, G, P).transpose(2, 1, 0)
        ).astype(NP_FP8)                                         # [p, g, t]
        smu = np.concatenate([s_all[i * T:(i + 1) * T], mu]).astype(
            NP_BF16
        ).reshape(1, T + RANK)
        in_maps.append({
            "x8": x8, "q8": q8, "rp8": rp8, "lT": lT, "smu": smu,
        })
    return in_maps


_NC_CACHE = {}


def _get_nc():
    if "nc" not in _NC_CACHE:
        _NC_CACHE["nc"] = build_nc()
    return _NC_CACHE["nc"]


def run(inputs, trace=False, tmpdir=None):
    nc = _get_nc()
    in_maps = make_in_maps(**inputs)
    res = run_bass_kernel_spmd(
        nc, in_maps, list(range(N_CORES)), trace=trace, tmpdir=tmpdir
    )
    shards = [
        np.asarray(res.results[i]["out"]).astype(np.float32)
        for i in range(N_CORES)
    ]
    full = np.concatenate(shards, axis=0)
    full += np.asarray(inputs["bias"], dtype=np.float32)
    return full.reshape(B, S, D_OUT), res


def kernel(**inputs) -> np.ndarray:
    out, _ = run(inputs, trace=False)
    return out


# revision 47
# speedup vs baseline: 1.0137x; 1.0070x over previous
"""CalderaLinear Trainium2 kernel (all-fp8 DMA + fp8 DoubleRow xr phase).

Computes out = x @ dequant(q).T + (x @ dequant(r).T) @ dequant(l).T + bias
with groupwise (group=128) dequantization, distributed over 8 NeuronCores
by sharding tokens (batch*seq) 8 ways and replicating the weights.

Numerics: the output scale is dominated by the low-rank path (|out| up to
~1.4e6), whose dominant component is rank-1-ish: c[n] = sum_k x[n,k]
amplified by r's positive mean. Quantizing x to fp8 puts ~3.6% noise on
c[n], so r is mean-centered on the host (r = r' + mu_j): the device
computes x8 @ r'8.T in fp8 DoubleRow (zero-mean r' kills the common-mode
amplification) and adds the rank-1 term mu_j * s[n] back via a K=1
matmul, with s[n] = sum_k x[n,k] computed exactly on the host. The xr
result and l stay bf16 for the closer GEMM (fp8 l puts a fixed per-output
error pattern on the large common component: measured 1.9e-2 absmax).
Simulated pipeline error: 6.0e-3 absmax (gate 2e-2).

Host does layout only: dequant-multiply + mean-center + transposes +
fp8/bf16 casts + per-token sums + token sharding.

Device per core (1024 tokens):
  x arrives directly as fp8 [P, G, T] (g-major), fetched in g-pair slices
  so the first output-block pair's matmuls start after ~2 DMA slices
  instead of the full tensor. The startup phase runs gp-major across
  token tiles 0-2 (6 PSUM banks) so each 512KB of DMA unlocks 6 matmuls.
  A short burst of N=64 warmup matmuls on a zeroed tile covers the
  initial DMA window and lifts the PE HAM clock-gate (1.2->2.4 GHz)
  before real work lands. Engines execute in emission order, so emission
  order here IS the schedule.
"""

import os
import sys

import numpy as np
import ml_dtypes

for _p in ("/opt/trn_rl_repo",):
    if _p not in sys.path and os.path.isdir(_p):
        sys.path.insert(0, _p)

import concourse.bass as bass
import concourse.mybir as mybir
import concourse.tile as tile
from concourse import bacc
from concourse.bass_utils import run_bass_kernel_spmd

BF16 = mybir.dt.bfloat16
F32 = mybir.dt.float32
FP8 = mybir.dt.float8e4
NP_FP8 = ml_dtypes.float8_e4m3
NP_BF16 = ml_dtypes.bfloat16

P = 128  # partitions / dequant group size
N_CORES = 8

# Full problem shape (hardcoded per contest contract).
B, S, D_IN, D_OUT, RANK = 4, 2048, 4096, 4096, 256
N_TOK = B * S          # 8192
T = N_TOK // N_CORES   # 1024 tokens per core
G = D_IN // P          # 32 k-chunks
GP = G // 2            # 16 k-pair-chunks (DoubleRow)
OBW = 512              # output block width
NOB = D_OUT // OBW     # 8 output blocks
RC = RANK // P         # 2 rank chunks
NT = T // P            # 8 token tiles
N_WARMUP = 64          # HAM warmup matmuls (N=64 each) before first data
# Warmup bursts woven between early gp-steps: insurance against DMA
# hiccups re-throttling the HAM clock-gate; taper to zero once DMA leads.
# Only gp 0-3 may weave warmups: they share xr1's PSUM bank, whose real
# accumulation (start=True) begins at gp==4.
WARM_WEAVE = [8, 6, 4, 2] + [0] * 12


def caldera_kernel(tc, out, x8_d, q8_d, rp8_d, lT_d, smu_d):
    """One core. DRAM tensors:
    x8_d    [P, G, T]            fp8   x8[p,g,t] = x[t, g*128+p]
    q8_d    [NOB, 128, GP*2*OBW] fp8   q8[ob,p,(gp,i,o)] =
                                         qdeq[(2gp+i)*128+p, ob*512+o]
    rp8_d   [P, G, RANK]         fp8   rp8[p,g,r] = (rdeq-mu)[r, g*128+p]
    lT_d    [128, RC, D_OUT]     bf16  lT[p,c,o] = ldeq[o, c*128+p]
    smu_d   [1, T+RANK]          bf16  [s[0:T] | mu[0:RANK]]
    out     [T, D_OUT]           bf16  (bias added on host)
    """
    nc = tc.nc
    DR = mybir.MatmulPerfMode.DoubleRow

    with tc.tile_pool(name="const", bufs=1) as constp, \
         tc.tile_pool(name="qsE", bufs=3) as qsE, \
         tc.tile_pool(name="outp", bufs=6) as outp, \
         tc.tile_pool(name="otwp", bufs=1) as otwp, \
         tc.tile_pool(name="ps", bufs=8, space="PSUM") as psp:

        # ---- resident tensors ----
        x8 = constp.tile([P, G, T], FP8)
        rp8 = constp.tile([P, G, RANK], FP8)
        lT = constp.tile([P, RC, D_OUT], BF16)
        xrT = constp.tile([P, RC, T], BF16)     # xr.T chunks (bf16)
        smu = constp.tile([P, T + RANK], BF16)  # row 0 only: [s | mu]
        warm = constp.tile([P, P], FP8)         # HAM warmup garbage

        qtiles = {}

        def fetch_q(ob, pool, frac=None):
            if ob not in qtiles:
                qtiles[ob] = pool.tile(
                    [P, GP, 2, OBW], FP8, tag="q8b", name=f"q8b{ob}"
                )
            qt = qtiles[ob]
            flat = qt[:].rearrange("p a b c -> p (a b c)")
            if frac is None:
                nc.sync.dma_start(out=flat, in_=q8_d[ob])
            else:
                i, n = frac
                h = GP * 2 * OBW // n
                nc.sync.dma_start(
                    out=flat[:, i * h:(i + 1) * h],
                    in_=q8_d[ob][:, i * h:(i + 1) * h],
                )

        def fetch_x(glo, ghi):
            nc.sync.dma_start(
                out=x8[:, glo:ghi, :], in_=x8_d[:, glo:ghi, :],
            )

        def open_gp(ps_pair, t, qA, qB, gp):
            """One DoubleRow k-pair step of the main GEMM for tile t."""
            psA, psB = ps_pair
            lhs = x8[:, 2 * gp:2 * gp + 2, t * P:(t + 1) * P]
            nc.tensor.matmul(
                psA[:], lhsT=lhs, rhs=qA[:, gp],
                start=(gp == 0), stop=False, perf_mode=DR,
            )
            nc.tensor.matmul(
                psB[:], lhsT=lhs, rhs=qB[:, gp],
                start=(gp == 0), stop=False, perf_mode=DR,
            )

        def main_group_open(ps_pair, t, qA, qB):
            for gp in range(GP):
                open_gp(ps_pair, t, qA, qB, gp)

        def open_solo_gp(ps, t, qt, gp):
            nc.tensor.matmul(
                ps[:], lhsT=x8[:, 2 * gp:2 * gp + 2, t * P:(t + 1) * P],
                rhs=qt[:, gp], start=(gp == 0), stop=False, perf_mode=DR,
            )

        def close_solo(ps, t, ob, fuse_store=False):
            """Closers + drain + store for a single (t, ob) group."""
            for rb in range(RC):
                nc.tensor.matmul(
                    ps[:], lhsT=xrT[:, rb, t * P:(t + 1) * P],
                    rhs=lT[:, rb, ob * OBW:(ob + 1) * OBW],
                    start=False, stop=(rb == RC - 1),
                )
            ot = outp.tile([P, OBW], BF16, tag="ot")
            if drain_tog[0] % 2 == 0:
                nc.scalar.copy(ot[:], ps[:])
            else:
                nc.vector.tensor_copy(out=ot[:], in_=ps[:])
            drain_tog[0] += 1
            nc.sync.dma_start(
                out=out[t * P:(t + 1) * P, ob * OBW:(ob + 1) * OBW],
                in_=ot[:],
            )

        drain_tog = [0]

        def close_fused(ps_pair, t, obA, obB):
            """Final-group close: both obs drain (scalar ∥ vector) into
            one wide tile, stored with a single DMA issue."""
            psA, psB = ps_pair
            for ps in (psA, psB):
                ob = obA if ps is psA else obB
                for rb in range(RC):
                    nc.tensor.matmul(
                        ps[:], lhsT=xrT[:, rb, t * P:(t + 1) * P],
                        rhs=lT[:, rb, ob * OBW:(ob + 1) * OBW],
                        start=False, stop=(rb == RC - 1),
                    )
            otw = otwp.tile([P, 2 * OBW], BF16, tag="otw")
            nc.scalar.copy(otw[:, 0:OBW], psA[:])
            nc.vector.tensor_copy(out=otw[:, OBW:2 * OBW], in_=psB[:])
            nc.sync.dma_start(
                out=out[t * P:(t + 1) * P, obA * OBW:(obB + 1) * OBW],
                in_=otw[:],
            )

        def main_group_close(ps_pair, t, obA, obB):
            """Low-rank closers (bf16) + psum drain + store.

            Bias is added on the host, so the drain is a plain
            fp32->bf16 copy; alternating it between the Scalar and
            Vector engines halves the serialized drain chain."""
            psA, psB = ps_pair
            for ps, ob in ((psA, obA), (psB, obB)):
                for rb in range(RC):
                    nc.tensor.matmul(
                        ps[:], lhsT=xrT[:, rb, t * P:(t + 1) * P],
                        rhs=lT[:, rb, ob * OBW:(ob + 1) * OBW],
                        start=False, stop=(rb == RC - 1),
                    )
                ot = outp.tile([P, OBW], BF16, tag="ot")
                if drain_tog[0] % 2 == 0:
                    nc.scalar.copy(ot[:], ps[:])
                else:
                    nc.vector.tensor_copy(out=ot[:], in_=ps[:])
                drain_tog[0] += 1
                nc.sync.dma_start(
                    out=out[t * P:(t + 1) * P, ob * OBW:(ob + 1) * OBW],
                    in_=ot[:],
                )

        def new_pair(name):
            psA = psp.tile([P, OBW], F32, tag="ps", name=f"psA{name}")
            psB = psp.tile([P, OBW], F32, tag="ps", name=f"psB{name}")
            return psA, psB

        def xr_alloc(th):
            return [
                psp.tile([P, OBW], F32, tag="ps", name=f"xrps{th}_{rb}")
                for rb in range(RC)
            ]

        def xr_open(pss, th, gplo, gphi):
            """xr accumulation (fp8 DoubleRow) for k-pairs [gplo, gphi)."""
            tok = slice(th * OBW, (th + 1) * OBW)
            for gp in range(gplo, gphi):
                for rb in range(RC):
                    nc.tensor.matmul(
                        pss[rb][:],
                        lhsT=rp8[:, 2 * gp:2 * gp + 2, rb * P:(rb + 1) * P],
                        rhs=x8[:, 2 * gp:2 * gp + 2, tok],
                        start=(gp == 0), stop=False, perf_mode=DR,
                    )

        def xr_close(pss, th):
            """Rank-1 mu[j]*s[n] term (K=1 bf16 matmul) + psum->bf16."""
            tok = slice(th * OBW, (th + 1) * OBW)
            for rb in range(RC):
                nc.tensor.matmul(
                    pss[rb][:],
                    lhsT=smu[0:1, T + rb * P:T + (rb + 1) * P],
                    rhs=smu[0:1, tok],
                    start=False, stop=True,
                )
            for rb in range(RC):
                nc.scalar.copy(xrT[:, rb, tok], pss[rb][:])

        # ================= DMA emission order = fetch priority ============
        # Each dma_start costs ~650ns of serial issue time on the Sync
        # engine, so effective bandwidth is proportional to DMA size for
        # small transfers. Use fine slices only for the first two
        # gp-steps (fast first matmul), then 512KB-1MB chunks in strict
        # consumption order. Bulk tensors (rp8, smu, lT, biasr, q2) slot
        # in after the gp-steps they'd otherwise delay.
        def fetch_rp8(qlo, qhi):
            nc.sync.dma_start(
                out=rp8[:, qlo:qhi, :], in_=rp8_d[:, qlo:qhi, :],
            )

        fetch_x(0, 2)                     # gp 0
        fetch_q(0, qsE, frac=(0, 16))
        fetch_x(2, 4)                     # gp 1
        fetch_q(0, qsE, frac=(1, 16))
        fetch_rp8(0, 8)
        fetch_x(4, 8)                     # gp 2-3
        fetch_q(0, qsE, frac=(1, 8))
        fetch_rp8(8, 16)
        fetch_x(8, 16)                    # gp 4-7
        fetch_q(0, qsE, frac=(1, 4))
        fetch_rp8(16, 24)
        fetch_x(16, 24)                   # gp 8-11
        fetch_q(0, qsE, frac=(2, 4))
        fetch_rp8(24, 32)
        fetch_x(24, 32)                   # gp 12-15
        fetch_q(0, qsE, frac=(3, 4))
        nc.sync.dma_start(out=smu[0:1, :], in_=smu_d[:])
        nc.sync.dma_start(out=lT[:, :, 0:2 * OBW], in_=lT_d[:, :, 0:2 * OBW])
        fetch_q(1, qsE)
        nc.sync.dma_start(
            out=lT[:, :, 2 * OBW:D_OUT], in_=lT_d[:, :, 2 * OBW:D_OUT]
        )
        fetch_q(2, qsE)

        # ================= PE emission order = schedule ===================
        # HAM warmup: keep the PE busy through the first DMA window.
        # Vector engine is ready ~2us before GpSimd, so memset there.
        # Warmup matmuls borrow xr0's PSUM bank (set below) — its real
        # accumulation later opens with start=True, clearing them.
        nc.vector.memset(warm[:], 0.0)
        wps = None

        def warmup(n):
            for _ in range(n):
                nc.tensor.matmul(
                    wps[:, 0:64], lhsT=warm[:, 0:P], rhs=warm[:, 0:64],
                    start=True, stop=True,
                )

        # ===== phase 1: ob0 solo groups for tiles 0-3, both xr halves ====
        # Solo groups cost the same per matmul as pairs (bass emits one
        # LDWEIGHTS per matmul regardless), but phase 1 then needs only
        # q0 (2MB) in the DMA-bound startup window. Tiles 0-3 (4 banks)
        # + xr0 + xr1 (4 banks) saturate the window with real work; the
        # warmups borrow xr1's bank before its gp==4 start=True clear.
        q0t = qtiles[0]
        solos = {t: psp.tile([P, OBW], F32, tag="ps", name=f"s0_{t}")
                 for t in range(4)}
        xr0 = xr_alloc(0)
        xr1 = xr_alloc(1)
        wps = xr1[0]
        warmup(N_WARMUP)
        for gp in range(GP):
            for t in range(4):
                open_solo_gp(solos[t], t, q0t, gp)
            warmup(WARM_WEAVE[gp])
            if gp >= 2:
                xr_open(xr0, 0, gp - 2, gp - 1)
            if gp >= 4:
                xr_open(xr1, 1, gp - 4, gp - 3)
        xr_open(xr0, 0, GP - 2, GP)
        xr_close(xr0, 0)
        xr_open(xr1, 1, GP - 4, GP)
        xr_close(xr1, 1)
        for t in range(4):
            close_solo(solos[t], t, 0)
        solos2 = {t: psp.tile([P, OBW], F32, tag="ps", name=f"s0_{t}")
                  for t in range(4, NT)}
        for t in range(4, NT):
            for gp in range(GP):
                open_solo_gp(solos2[t], t, q0t, gp)
        for t in range(4, NT):
            close_solo(solos2[t], t, 0)

        # ======== phase 2: ob pairs (1,2),(3,4),(5,6), then ob7 =========
        # Batch opens before closes: each DoubleRow<->bf16 PE mode switch
        # costs ~200ns on the first matmul after it, so group the bf16
        # closers of several tiles together (PSUM budget permitting).
        with tc.tile_pool(name="qsL", bufs=5) as qsL:
            for ob in range(3, NOB):
                fetch_q(ob, qsL)
            for obA in (1, 3, 5):
                obB = obA + 1
                qA, qB = qtiles[obA], qtiles[obB]
                for tb in ((0, 1, 2, 3), (4, 5, 6, 7)):
                    pps = [new_pair(f"{obA}_{t}") for t in tb]
                    for i, t in enumerate(tb):
                        main_group_open(pps[i], t, qA, qB)
                    for i, t in enumerate(tb):
                        main_group_close(pps[i], t, obA, obB)
            # ob7 solo, tapered so only one drain+store lands after the
            # last matmul.
            q7t = qtiles[7]
            for tb in ((0, 1, 2, 3, 4, 5), (6, 7)):
                ss = {t: psp.tile([P, OBW], F32, tag="ps", name=f"s7_{t}")
                      for t in tb}
                for t in tb:
                    for gp in range(GP):
                        open_solo_gp(ss[t], t, q7t, gp)
                for t in tb:
                    close_solo(ss[t], t, NOB - 1)


def build_nc():
    nc = bacc.Bacc("TRN2", target_bir_lowering=False, debug=False)
    x8_d = nc.dram_tensor("x8", [P, G, T], FP8, kind="ExternalInput").ap()
    q8_d = nc.dram_tensor(
        "q8", [NOB, P, GP * 2 * OBW], FP8, kind="ExternalInput"
    ).ap()
    rp8_d = nc.dram_tensor(
        "rp8", [P, G, RANK], FP8, kind="ExternalInput"
    ).ap()
    lT_d = nc.dram_tensor("lT", [P, RC, D_OUT], BF16, kind="ExternalInput").ap()
    smu_d = nc.dram_tensor(
        "smu", [1, T + RANK], BF16, kind="ExternalInput"
    ).ap()
    out = nc.dram_tensor("out", [T, D_OUT], BF16, kind="ExternalOutput").ap()
    with tile.TileContext(nc) as tc:
        caldera_kernel(tc, out, x8_d, q8_d, rp8_d, lT_d, smu_d)
    nc.compile()
    return nc


def _dequant(vals, scales):
    rows, cols = vals.shape
    g = cols // P
    v = vals.astype(np.float32).reshape(rows, g, P) * scales[:, :, None]
    return v.reshape(rows, cols)


def make_in_maps(x, q_values, q_scales, l_values, l_scales, r_values, r_scales,
                 bias):
    # q: dequant -> [k, o] transpose -> fp8, packed per 512-col block:
    # q8[ob, p, (gp, i, o)] = qdeq[(2gp+i)*128+p, ob*512+o]
    qdeq = _dequant(np.asarray(q_values), np.asarray(q_scales))  # [o, k]
    qT = np.ascontiguousarray(qdeq.T).astype(NP_FP8)             # [k, o]
    q8 = qT.reshape(GP, 2, P, NOB, OBW).transpose(3, 2, 0, 1, 4)
    q8 = np.ascontiguousarray(q8).reshape(NOB, P, GP * 2 * OBW)

    rdeq = _dequant(np.asarray(r_values), np.asarray(r_scales))  # [r, k]
    mu = rdeq.mean(axis=1).astype(np.float32)                    # [r]
    rp = rdeq - mu[:, None]                                      # zero-mean
    rp8 = np.ascontiguousarray(
        rp.T.reshape(G, P, RANK).transpose(1, 0, 2)
    ).astype(NP_FP8)                                             # [p, g, r]

    ldeq = _dequant(np.asarray(l_values), np.asarray(l_scales))  # [o, r]
    lT = np.ascontiguousarray(
        ldeq.T.reshape(RC, P, D_OUT).transpose(1, 0, 2)
    ).astype(NP_BF16)                                            # [p, c, o]

    xf = np.asarray(x, dtype=np.float32).reshape(N_TOK, D_IN)
    s_all = xf.sum(axis=1)                                       # [n_tok]
    in_maps = []
    for i in range(N_CORES):
        xs = xf[i * T:(i + 1) * T]                               # [t, k]
        x8 = np.ascontiguousarray(
            xs.reshape(T, G, P).transpose(2, 1, 0)
        ).astype(NP_FP8)                                         # [p, g, t]
        smu = np.concatenate([s_all[i * T:(i + 1) * T], mu]).astype(
            NP_BF16
        ).reshape(1, T + RANK)
        in_maps.append({
            "x8": x8, "q8": q8, "rp8": rp8, "lT": lT, "smu": smu,
        })
    return in_maps


_NC_CACHE = {}


def _get_nc():
    if "nc" not in _NC_CACHE:
        _NC_CACHE["nc"] = build_nc()
    return _NC_CACHE["nc"]


def run(inputs, trace=False, tmpdir=None):
    nc = _get_nc()
    in_maps = make_in_maps(**inputs)
    res = run_bass_kernel_spmd(
        nc, in_maps, list(range(N_CORES)), trace=trace, tmpdir=tmpdir
    )
    shards = [
        np.asarray(res.results[i]["out"]).astype(np.float32)
        for i in range(N_CORES)
    ]
    full = np.concatenate(shards, axis=0)
    full += np.asarray(inputs["bias"], dtype=np.float32)
    return full.reshape(B, S, D_OUT), res


def kernel(**inputs) -> np.ndarray:
    out, _ = run(inputs, trace=False)
    return out


# revision 55
# speedup vs baseline: 1.0183x; 1.0044x over previous
"""CalderaLinear Trainium2 kernel (all-fp8 DMA + fp8 DoubleRow xr phase).

Computes out = x @ dequant(q).T + (x @ dequant(r).T) @ dequant(l).T + bias
with groupwise (group=128) dequantization, distributed over 8 NeuronCores
by sharding tokens (batch*seq) 8 ways and replicating the weights.

Numerics: the output scale is dominated by the low-rank path (|out| up to
~1.4e6), whose dominant component is rank-1-ish: c[n] = sum_k x[n,k]
amplified by r's positive mean. Quantizing x to fp8 puts ~3.6% noise on
c[n], so r is mean-centered on the host (r = r' + mu_j): the device
computes x8 @ r'8.T in fp8 DoubleRow (zero-mean r' kills the common-mode
amplification) and adds the rank-1 term mu_j * s[n] back in the fused
DVE drain, with s[n] = sum_k x[n,k] computed exactly on the host. The xr
result and l stay bf16 for the closer GEMM (fp8 l puts a fixed per-output
error pattern on the large common component: measured 1.9e-2 absmax).
Simulated pipeline error: 6.0e-3 absmax (gate 2e-2).

Host does layout only: dequant-multiply + mean-center + transposes +
fp8/bf16 casts + per-token sums + token sharding.

Device per core (1024 tokens):
  x arrives directly as fp8 [P, G, T] (g-major), fetched in g-pair slices
  so the first output-block pair's matmuls start after ~2 DMA slices
  instead of the full tensor. The startup phase runs gp-major across
  token tiles 0-2 (6 PSUM banks) so each 512KB of DMA unlocks 6 matmuls.
  A short burst of N=64 warmup matmuls on a zeroed tile covers the
  initial DMA window and lifts the PE HAM clock-gate (1.2->2.4 GHz)
  before real work lands. Engines execute in emission order, so emission
  order here IS the schedule.
"""

import os
import sys

import numpy as np
import ml_dtypes

for _p in ("/opt/trn_rl_repo",):
    if _p not in sys.path and os.path.isdir(_p):
        sys.path.insert(0, _p)

import concourse.bass as bass
import concourse.mybir as mybir
import concourse.tile as tile
from concourse import bacc
from concourse.bass_utils import run_bass_kernel_spmd

BF16 = mybir.dt.bfloat16
F32 = mybir.dt.float32
FP8 = mybir.dt.float8e4
NP_FP8 = ml_dtypes.float8_e4m3
NP_BF16 = ml_dtypes.bfloat16

P = 128  # partitions / dequant group size
N_CORES = 8

# Full problem shape (hardcoded per contest contract).
B, S, D_IN, D_OUT, RANK = 4, 2048, 4096, 4096, 256
N_TOK = B * S          # 8192
T = N_TOK // N_CORES   # 1024 tokens per core
G = D_IN // P          # 32 k-chunks
GP = G // 2            # 16 k-pair-chunks (DoubleRow)
OBW = 512              # output block width
NOB = D_OUT // OBW     # 8 output blocks
RC = RANK // P         # 2 rank chunks
NT = T // P            # 8 token tiles
N_WARMUP = 64          # HAM warmup matmuls (N=64 each) before first data
# Warmup bursts woven between early gp-steps: insurance against DMA
# hiccups re-throttling the HAM clock-gate; taper to zero once DMA leads.
# Only gp 0-3 may weave warmups: they share xr1's PSUM bank, whose real
# accumulation (start=True) begins at gp==4.
WARM_WEAVE = [8, 6, 4, 2] + [0] * 12


def caldera_kernel(tc, out, x8_d, q8_d, rp8_d, lT_d, smu_d):
    """One core. DRAM tensors:
    x8_d    [P, G, T]            fp8   x8[p,g,t] = x[t, g*128+p]
    q8_d    [NOB, 128, GP*2*OBW] fp8   q8[ob,p,(gp,i,o)] =
                                         qdeq[(2gp+i)*128+p, ob*512+o]
    rp8_d   [P, G, RANK]         fp8   rp8[p,g,r] = (rdeq-mu)[r, g*128+p]
    lT_d    [128, RC, D_OUT]     bf16  lT[p,c,o] = ldeq[o, c*128+p]
    smu_d   [1, T+RANK]          bf16  [s[0:T] | mu[0:RANK]]
    out     [T, D_OUT]           bf16  (bias added on host)
    """
    nc = tc.nc
    DR = mybir.MatmulPerfMode.DoubleRow

    with tc.tile_pool(name="const", bufs=1) as constp, \
         tc.tile_pool(name="qsE", bufs=3) as qsE, \
         tc.tile_pool(name="outp", bufs=6) as outp, \
         tc.tile_pool(name="otwp", bufs=1) as otwp, \
         tc.tile_pool(name="ps", bufs=8, space="PSUM") as psp:

        # ---- resident tensors ----
        x8 = constp.tile([P, G, T], FP8)
        rp8 = constp.tile([P, G, RANK], FP8)
        lT = constp.tile([P, RC, D_OUT], BF16)
        xrT = constp.tile([P, RC, T], BF16)     # xr.T chunks (bf16)
        # [s broadcast over partitions | mu by rank chunk]:
        # smu[p, 0:T] = s[t], smu[p, T+rb] = mu[rb*128+p]
        smu = constp.tile([P, T + RC], BF16)
        warm = constp.tile([P, P], FP8)         # HAM warmup garbage

        qtiles = {}

        def fetch_q(ob, pool, frac=None):
            if ob not in qtiles:
                qtiles[ob] = pool.tile(
                    [P, GP, 2, OBW], FP8, tag="q8b", name=f"q8b{ob}"
                )
            qt = qtiles[ob]
            flat = qt[:].rearrange("p a b c -> p (a b c)")
            if frac is None:
                nc.sync.dma_start(out=flat, in_=q8_d[ob])
            else:
                i, n = frac
                h = GP * 2 * OBW // n
                nc.sync.dma_start(
                    out=flat[:, i * h:(i + 1) * h],
                    in_=q8_d[ob][:, i * h:(i + 1) * h],
                )

        def fetch_x(glo, ghi):
            nc.sync.dma_start(
                out=x8[:, glo:ghi, :], in_=x8_d[:, glo:ghi, :],
            )

        def open_gp(ps_pair, t, qA, qB, gp):
            """One DoubleRow k-pair step of the main GEMM for tile t."""
            psA, psB = ps_pair
            lhs = x8[:, 2 * gp:2 * gp + 2, t * P:(t + 1) * P]
            nc.tensor.matmul(
                psA[:], lhsT=lhs, rhs=qA[:, gp],
                start=(gp == 0), stop=False, perf_mode=DR,
            )
            nc.tensor.matmul(
                psB[:], lhsT=lhs, rhs=qB[:, gp],
                start=(gp == 0), stop=False, perf_mode=DR,
            )

        def main_group_open(ps_pair, t, qA, qB):
            for gp in range(GP):
                open_gp(ps_pair, t, qA, qB, gp)

        def open_solo_gp(ps, t, qt, gp):
            nc.tensor.matmul(
                ps[:], lhsT=x8[:, 2 * gp:2 * gp + 2, t * P:(t + 1) * P],
                rhs=qt[:, gp], start=(gp == 0), stop=False, perf_mode=DR,
            )

        def close_solo(ps, t, ob, fuse_store=False):
            """Closers + drain + store for a single (t, ob) group."""
            for rb in range(RC):
                nc.tensor.matmul(
                    ps[:], lhsT=xrT[:, rb, t * P:(t + 1) * P],
                    rhs=lT[:, rb, ob * OBW:(ob + 1) * OBW],
                    start=False, stop=(rb == RC - 1),
                )
            ot = outp.tile([P, OBW], BF16, tag="ot")
            if drain_tog[0] % 2 == 0:
                nc.scalar.copy(ot[:], ps[:])
            else:
                nc.vector.tensor_copy(out=ot[:], in_=ps[:])
            drain_tog[0] += 1
            nc.sync.dma_start(
                out=out[t * P:(t + 1) * P, ob * OBW:(ob + 1) * OBW],
                in_=ot[:],
            )

        drain_tog = [0]

        def close_fused(ps_pair, t, obA, obB):
            """Final-group close: both obs drain (scalar ∥ vector) into
            one wide tile, stored with a single DMA issue."""
            psA, psB = ps_pair
            for ps in (psA, psB):
                ob = obA if ps is psA else obB
                for rb in range(RC):
                    nc.tensor.matmul(
                        ps[:], lhsT=xrT[:, rb, t * P:(t + 1) * P],
                        rhs=lT[:, rb, ob * OBW:(ob + 1) * OBW],
                        start=False, stop=(rb == RC - 1),
                    )
            otw = otwp.tile([P, 2 * OBW], BF16, tag="otw")
            nc.scalar.copy(otw[:, 0:OBW], psA[:])
            nc.vector.tensor_copy(out=otw[:, OBW:2 * OBW], in_=psB[:])
            nc.sync.dma_start(
                out=out[t * P:(t + 1) * P, obA * OBW:(obB + 1) * OBW],
                in_=otw[:],
            )

        def main_group_close(ps_pair, t, obA, obB):
            """Low-rank closers (bf16) + psum drain + store.

            Bias is added on the host, so the drain is a plain
            fp32->bf16 copy; alternating it between the Scalar and
            Vector engines halves the serialized drain chain."""
            psA, psB = ps_pair
            for ps, ob in ((psA, obA), (psB, obB)):
                for rb in range(RC):
                    nc.tensor.matmul(
                        ps[:], lhsT=xrT[:, rb, t * P:(t + 1) * P],
                        rhs=lT[:, rb, ob * OBW:(ob + 1) * OBW],
                        start=False, stop=(rb == RC - 1),
                    )
                ot = outp.tile([P, OBW], BF16, tag="ot")
                if drain_tog[0] % 2 == 0:
                    nc.scalar.copy(ot[:], ps[:])
                else:
                    nc.vector.tensor_copy(out=ot[:], in_=ps[:])
                drain_tog[0] += 1
                nc.sync.dma_start(
                    out=out[t * P:(t + 1) * P, ob * OBW:(ob + 1) * OBW],
                    in_=ot[:],
                )

        def new_pair(name):
            psA = psp.tile([P, OBW], F32, tag="ps", name=f"psA{name}")
            psB = psp.tile([P, OBW], F32, tag="ps", name=f"psB{name}")
            return psA, psB

        def xr_alloc(th):
            return [
                psp.tile([P, OBW], F32, tag="ps", name=f"xrps{th}_{rb}")
                for rb in range(RC)
            ]

        def xr_open(pss, th, gplo, gphi, final=False):
            """xr accumulation (fp8 DoubleRow) for k-pairs [gplo, gphi)."""
            tok = slice(th * OBW, (th + 1) * OBW)
            for gp in range(gplo, gphi):
                for rb in range(RC):
                    nc.tensor.matmul(
                        pss[rb][:],
                        lhsT=rp8[:, 2 * gp:2 * gp + 2, rb * P:(rb + 1) * P],
                        rhs=x8[:, 2 * gp:2 * gp + 2, tok],
                        start=(gp == 0), stop=(final and gp == gphi - 1),
                        perf_mode=DR,
                    )

        def xr_close(pss, th):
            """Fused drain: xrT = mu[rank] * s[tok] + psum (one DVE op
            per rank chunk; mu is a per-partition scalar here)."""
            tok = slice(th * OBW, (th + 1) * OBW)
            for rb in range(RC):
                nc.vector.scalar_tensor_tensor(
                    out=xrT[:, rb, tok], in0=smu[:, tok],
                    scalar=smu[:, T + rb:T + rb + 1], in1=pss[rb][:],
                    op0=mybir.AluOpType.mult, op1=mybir.AluOpType.add,
                )

        # ================= DMA emission order = fetch priority ============
        # Each dma_start costs ~650ns of serial issue time on the Sync
        # engine, so effective bandwidth is proportional to DMA size for
        # small transfers. Use fine slices only for the first two
        # gp-steps (fast first matmul), then 512KB-1MB chunks in strict
        # consumption order. Bulk tensors (rp8, smu, lT, biasr, q2) slot
        # in after the gp-steps they'd otherwise delay.
        def fetch_rp8(qlo, qhi):
            nc.sync.dma_start(
                out=rp8[:, qlo:qhi, :], in_=rp8_d[:, qlo:qhi, :],
            )

        fetch_x(0, 2)                     # gp 0
        fetch_q(0, qsE, frac=(0, 16))
        fetch_x(2, 4)                     # gp 1
        fetch_q(0, qsE, frac=(1, 16))
        fetch_rp8(0, 8)
        fetch_x(4, 8)                     # gp 2-3
        fetch_q(0, qsE, frac=(1, 8))
        fetch_rp8(8, 16)
        fetch_x(8, 16)                    # gp 4-7
        fetch_q(0, qsE, frac=(1, 4))
        fetch_rp8(16, 24)
        fetch_x(16, 24)                   # gp 8-11
        fetch_q(0, qsE, frac=(2, 4))
        fetch_rp8(24, 32)
        fetch_x(24, 32)                   # gp 12-15
        fetch_q(0, qsE, frac=(3, 4))
        nc.sync.dma_start(out=smu[:], in_=smu_d[:])
        nc.sync.dma_start(out=lT[:, :, 0:2 * OBW], in_=lT_d[:, :, 0:2 * OBW])
        fetch_q(1, qsE)
        nc.sync.dma_start(
            out=lT[:, :, 2 * OBW:D_OUT], in_=lT_d[:, :, 2 * OBW:D_OUT]
        )
        fetch_q(2, qsE)

        # ================= PE emission order = schedule ===================
        # HAM warmup: keep the PE busy through the first DMA window.
        # Vector engine is ready ~2us before GpSimd, so memset there.
        # Warmup matmuls borrow xr0's PSUM bank (set below) — its real
        # accumulation later opens with start=True, clearing them.
        nc.vector.memset(warm[:], 0.0)
        wps = None

        def warmup(n):
            for _ in range(n):
                nc.tensor.matmul(
                    wps[:, 0:64], lhsT=warm[:, 0:P], rhs=warm[:, 0:64],
                    start=True, stop=True,
                )

        # ===== phase 1: ob0 solo groups for tiles 0-3, both xr halves ====
        # Solo groups cost the same per matmul as pairs (bass emits one
        # LDWEIGHTS per matmul regardless), but phase 1 then needs only
        # q0 (2MB) in the DMA-bound startup window. Tiles 0-3 (4 banks)
        # + xr0 + xr1 (4 banks) saturate the window with real work; the
        # warmups borrow xr1's bank before its gp==4 start=True clear.
        q0t = qtiles[0]
        solos = {t: psp.tile([P, OBW], F32, tag="ps", name=f"s0_{t}")
                 for t in range(4)}
        xr0 = xr_alloc(0)
        xr1 = xr_alloc(1)
        wps = xr1[0]
        warmup(N_WARMUP)
        # Staggered lags (t2/t3 two gp-steps behind, xr0 four, xr1 six)
        # turn each arriving DMA slice into a backlog of runnable work,
        # so the early bandwidth ramp doesn't stall the PE.
        for gp in range(GP):
            open_solo_gp(solos[0], 0, q0t, gp)
            open_solo_gp(solos[1], 1, q0t, gp)
            warmup(WARM_WEAVE[gp])
            if gp >= 2:
                open_solo_gp(solos[2], 2, q0t, gp - 2)
                open_solo_gp(solos[3], 3, q0t, gp - 2)
            if gp >= 4:
                xr_open(xr0, 0, gp - 4, gp - 3)
            if gp >= 6:
                xr_open(xr1, 1, gp - 6, gp - 5)
        for gp in (GP - 2, GP - 1):
            open_solo_gp(solos[2], 2, q0t, gp)
            open_solo_gp(solos[3], 3, q0t, gp)
        xr_open(xr0, 0, GP - 4, GP, final=True)
        xr_close(xr0, 0)
        xr_open(xr1, 1, GP - 6, GP, final=True)
        xr_close(xr1, 1)
        for t in range(4):
            close_solo(solos[t], t, 0)
        solos2 = {t: psp.tile([P, OBW], F32, tag="ps", name=f"s0_{t}")
                  for t in range(4, NT)}
        for t in range(4, NT):
            for gp in range(GP):
                open_solo_gp(solos2[t], t, q0t, gp)
        for t in range(4, NT):
            close_solo(solos2[t], t, 0)

        # ======== phase 2: ob pairs (1,2),(3,4),(5,6), then ob7 =========
        # Batch opens before closes: each DoubleRow<->bf16 PE mode switch
        # costs ~200ns on the first matmul after it, so group the bf16
        # closers of several tiles together (PSUM budget permitting).
        with tc.tile_pool(name="qsL", bufs=5) as qsL:
            for ob in range(3, NOB):
                fetch_q(ob, qsL)
            for obA in (1, 3, 5):
                obB = obA + 1
                qA, qB = qtiles[obA], qtiles[obB]
                for tb in ((0, 1, 2, 3), (4, 5, 6, 7)):
                    pps = [new_pair(f"{obA}_{t}") for t in tb]
                    for i, t in enumerate(tb):
                        main_group_open(pps[i], t, qA, qB)
                    for i, t in enumerate(tb):
                        main_group_close(pps[i], t, obA, obB)
            # ob7 solo, tapered so only one drain+store lands after the
            # last matmul.
            q7t = qtiles[7]
            for tb in ((0, 1, 2, 3, 4, 5), (6, 7)):
                ss = {t: psp.tile([P, OBW], F32, tag="ps", name=f"s7_{t}")
                      for t in tb}
                for t in tb:
                    for gp in range(GP):
                        open_solo_gp(ss[t], t, q7t, gp)
                for t in tb:
                    close_solo(ss[t], t, NOB - 1)


def build_nc():
    nc = bacc.Bacc("TRN2", target_bir_lowering=False, debug=False)
    x8_d = nc.dram_tensor("x8", [P, G, T], FP8, kind="ExternalInput").ap()
    q8_d = nc.dram_tensor(
        "q8", [NOB, P, GP * 2 * OBW], FP8, kind="ExternalInput"
    ).ap()
    rp8_d = nc.dram_tensor(
        "rp8", [P, G, RANK], FP8, kind="ExternalInput"
    ).ap()
    lT_d = nc.dram_tensor("lT", [P, RC, D_OUT], BF16, kind="ExternalInput").ap()
    smu_d = nc.dram_tensor(
        "smu", [P, T + RC], BF16, kind="ExternalInput"
    ).ap()
    out = nc.dram_tensor("out", [T, D_OUT], BF16, kind="ExternalOutput").ap()
    with tile.TileContext(nc) as tc:
        caldera_kernel(tc, out, x8_d, q8_d, rp8_d, lT_d, smu_d)
    nc.compile()
    return nc


def _dequant(vals, scales):
    rows, cols = vals.shape
    g = cols // P
    v = vals.astype(np.float32).reshape(rows, g, P) * scales[:, :, None]
    return v.reshape(rows, cols)


def make_in_maps(x, q_values, q_scales, l_values, l_scales, r_values, r_scales,
                 bias):
    # q: dequant -> [k, o] transpose -> fp8, packed per 512-col block:
    # q8[ob, p, (gp, i, o)] = qdeq[(2gp+i)*128+p, ob*512+o]
    qdeq = _dequant(np.asarray(q_values), np.asarray(q_scales))  # [o, k]
    qT = np.ascontiguousarray(qdeq.T).astype(NP_FP8)             # [k, o]
    q8 = qT.reshape(GP, 2, P, NOB, OBW).transpose(3, 2, 0, 1, 4)
    q8 = np.ascontiguousarray(q8).reshape(NOB, P, GP * 2 * OBW)

    rdeq = _dequant(np.asarray(r_values), np.asarray(r_scales))  # [r, k]
    mu = rdeq.mean(axis=1).astype(np.float32)                    # [r]
    rp = rdeq - mu[:, None]                                      # zero-mean
    rp8 = np.ascontiguousarray(
        rp.T.reshape(G, P, RANK).transpose(1, 0, 2)
    ).astype(NP_FP8)                                             # [p, g, r]

    ldeq = _dequant(np.asarray(l_values), np.asarray(l_scales))  # [o, r]
    lT = np.ascontiguousarray(
        ldeq.T.reshape(RC, P, D_OUT).transpose(1, 0, 2)
    ).astype(NP_BF16)                                            # [p, c, o]

    xf = np.asarray(x, dtype=np.float32).reshape(N_TOK, D_IN)
    s_all = xf.sum(axis=1)                                       # [n_tok]
    in_maps = []
    for i in range(N_CORES):
        xs = xf[i * T:(i + 1) * T]                               # [t, k]
        x8 = np.ascontiguousarray(
            xs.reshape(T, G, P).transpose(2, 1, 0)
        ).astype(NP_FP8)                                         # [p, g, t]
        s_b = np.broadcast_to(
            s_all[i * T:(i + 1) * T][None, :], (P, T)
        )                                                        # [p, t]
        musc = mu.reshape(RC, P).T                               # [p, rb]
        smu = np.ascontiguousarray(
            np.concatenate([s_b, musc], axis=1)
        ).astype(NP_BF16)
        in_maps.append({
            "x8": x8, "q8": q8, "rp8": rp8, "lT": lT, "smu": smu,
        })
    return in_maps


_NC_CACHE = {}


def _get_nc():
    if "nc" not in _NC_CACHE:
        _NC_CACHE["nc"] = build_nc()
    return _NC_CACHE["nc"]


def run(inputs, trace=False, tmpdir=None):
    nc = _get_nc()
    in_maps = make_in_maps(**inputs)
    res = run_bass_kernel_spmd(
        nc, in_maps, list(range(N_CORES)), trace=trace, tmpdir=tmpdir
    )
    shards = [
        np.asarray(res.results[i]["out"]).astype(np.float32)
        for i in range(N_CORES)
    ]
    full = np.concatenate(shards, axis=0)
    full += np.asarray(inputs["bias"], dtype=np.float32)
    return full.reshape(B, S, D_OUT), res


def kernel(**inputs) -> np.ndarray:
    out, _ = run(inputs, trace=False)
    return out
